# revision 1
# baseline (speedup 1.0000x reference)
"""ChannelAttention kernel for Trainium2 (8 NeuronCores, batch-parallel).

Reference computation per batch element b (C=64, N=H*W=65536):
    X1 = x[b] viewed [C, N]          (proj_query)
    X2 = x[b] viewed [N, C]          (proj_key -- a reshape, NOT a transpose)
    S  = X1 @ X2                     [C, C]
    P  = softmax(S, axis=-1)
    out[b] = (P @ X1) + X1  =  (P + I) @ X1

Sharding: data-parallel over batch. B=16 -> 2 batches per core on 8 cores.

Per-core dataflow (per batch):
  - x[b] resident in SBUF as 16 column-strips [128, 2048] f32: partition c
    holds X1[c, colhalf0-window], partition 64+c holds X1[c, colhalf1-window].
  - mm1 lhsT tiles: PE-transpose of strip slices [128,128] -> X1^T tiles for
    two n-windows at once (cols 0:64 = window u, cols 64:128 = window u+256).
  - mm1 rhs tiles: X2 contraction-major tiles streamed from HBM with a
    strided access pattern ([128, 32, 64] chunks, 1MB contiguous reads).
  - S accumulates over 512 matmuls in one PSUM tile [64, 64].
  - softmax: DVE row-max (negated) -> ACT exp with fused row-sum ->
    DVE reciprocal -> fused (E * 1/sum) + I.
  - (P+I)^T via PE transpose; replicated to partitions 64:128 via tiny
    SBUF->SBUF DMA so both column-halves of mm2 have aligned operands.
  - mm2: 128 matmuls [64p x 64] @ [64p x 512] -> PSUM -> copy (DVE/ACT
    alternating) into [64, 2048] staging -> 512KB stores to HBM.
"""

import numpy as np

_CACHE = {}

B_FULL = 16
C = 64
N = 65536          # H*W = 256*256
NB = 2             # batches per core
NCORES = 8
NWIN = 256         # 128-col windows per column-half (32768 / 128)
NSTRIP = 16        # strips per batch; strip = [128, 2048]
STRIPW = 2048
VCH = 32           # X2 tiles per V chunk (1 MB per chunk)
NCHUNK = 8         # V chunk pairs per batch (8 * 32 = 256 windows)


def _build(loop_reps=1):
    from contextlib import nullcontext

    import concourse.bacc as bacc
    import concourse.mybir as mybir
    import concourse.tile as tile
    from concourse.masks import make_identity

    f32 = mybir.dt.float32
    Alu = mybir.AluOpType
    Act = mybir.ActivationFunctionType

    nc = bacc.Bacc("TRN2", debug=False)
    xb = nc.dram_tensor("xb", [NB, C * N], f32, kind="ExternalInput").ap()
    ob = nc.dram_tensor("ob", [NB, C * N], f32, kind="ExternalOutput").ap()

    with tile.TileContext(nc) as tc:
        with (
            tc.tile_pool(name="consts", bufs=1) as consts,
            tc.tile_pool(name="H", bufs=NSTRIP) as hpool,
            tc.tile_pool(name="V", bufs=3) as vpool,
            tc.tile_pool(name="TOs", bufs=6) as topool,
            tc.tile_pool(name="stage", bufs=2) as stpool,
            tc.tile_pool(name="soft", bufs=2) as softpool,
            tc.tile_pool(name="psT", bufs=3, space="PSUM") as psT,
            tc.tile_pool(name="psS", bufs=1, space="PSUM") as psS,
            tc.tile_pool(name="psO", bufs=2, space="PSUM") as psO,
            tc.tile_pool(name="psP", bufs=1, space="PSUM") as psP,
        ):
            ident = consts.tile([128, 128], f32)
            make_identity(nc, ident[:])

            loop_cm = (
                tc.For_i(0, loop_reps, 1) if loop_reps > 1 else nullcontext()
            )
            with loop_cm:
              for b in range(NB):
                x1 = xb[b].rearrange("(c n) -> c n", c=C)      # [64, 65536]
                o1 = ob[b].rearrange("(c n) -> c n", c=C)

                # ---- load phase: interleave H strips and V chunks ----
                # Each strip is loaded by two 64-partition DMAs placed on the
                # two HWDGE rings (sync/scalar): partitions 0-63 hit the even
                # SBUF AXI ports and 64-127 the odd ones, so the concurrent
                # pair covers all 16 ports.
                strips = []
                vtiles = []
                for j in range(NCHUNK):
                    for k in (2 * j, 2 * j + 1):
                        st = hpool.tile([128, STRIPW], f32, tag="H")
                        nc.sync.dma_start(
                            st[0:64, :], x1[:, k * STRIPW:(k + 1) * STRIPW]
                        )
                        nc.scalar.dma_start(
                            st[64:128, :],
                            x1[:, 32768 + k * STRIPW: 32768 + (k + 1) * STRIPW],
                        )
                        strips.append(st)
                    # V chunk pair: tiles t in [32j, 32j+32) and [256+32j, ...)
                    # interleaved as [128, tl, half, c] so window u's matmul
                    # can take rhs = [U_u | U_{u+256}] as one [128, 128] slice.
                    vt = vpool.tile([128, VCH, 2, C], f32, tag="V")
                    for half in range(2):
                        t0 = 256 * half + VCH * j
                        src = xb[b][t0 * 8192:(t0 + VCH) * 8192].rearrange(
                            "(t p c) -> p t c", p=128, c=C
                        )
                        nc.sync.dma_start(vt[:, :, half, :], src)
                    vtiles.append(vt)

                # ---- mm1: S = X1 @ X2, accumulated over 512 tiles ----
                # One N=128 matmul per window: lhsT = [T_u | T_{u+256}]
                # (from one transpose), rhs = [U_u | U_{u+256}].  The two
                # diagonal 64x64 blocks of the [128, 128] accumulator hold
                # the real contributions; off-diagonal blocks are ignored.
                # PE stream is software-pipelined: transposes for pair p+SKEW
                # are emitted before the matmuls of pair p, so the PE never
                # waits on the PSUM->SBUF copy chain.
                SKEW = 2
                s_ps = psS.tile([128, 128], f32, tag="S")
                npairs = NWIN // 2               # 128 window pairs
                to_sbs = {}

                def emit_transpose(tp):
                    u0 = 2 * tp
                    to_ps = psT.tile([128, 2, 128], f32, tag="TO")
                    for q in range(2):
                        u = u0 + q
                        st = strips[u // 16]
                        ti = st[:, (u % 16) * 128:(u % 16) * 128 + 128]
                        nc.tensor.transpose(to_ps[:, q, :], ti, ident[:])
                    to_sb = topool.tile([128, 2, 128], f32, tag="TOs")
                    if tp % 2 == 0:
                        nc.scalar.copy(to_sb[:], to_ps[:])
                    else:
                        nc.vector.tensor_copy(to_sb[:], to_ps[:])
                    to_sbs[tp] = to_sb

                for tp in range(SKEW):
                    emit_transpose(tp)
                for tp in range(npairs):
                    if tp + SKEW < npairs:
                        emit_transpose(tp + SKEW)
                    to_sb = to_sbs.pop(tp)
                    for q in range(2):
                        u = 2 * tp + q
                        j, tl = u // VCH, u % VCH
                        nc.tensor.matmul(
                            s_ps[:], to_sb[:, q, :],
                            vtiles[j][:, tl, :, :],
                            start=(u == 0), stop=(u == NWIN - 1),
                        )

                # ---- S = UL + LR (diagonal blocks of the accumulator) ----
                s_sb = softpool.tile([128, 128], f32, tag="Ssb")
                nc.vector.tensor_copy(s_sb[:], s_ps[:])
                s_fix = softpool.tile([64, 64], f32, tag="Sfix")
                nc.sync.dma_start(s_fix[:], s_sb[64:128, 64:128])
                s2_sb = softpool.tile([64, 64], f32, tag="S2")
                nc.vector.tensor_add(s2_sb[:], s_sb[0:64, 0:64], s_fix[:])

                # ---- softmax + (P + I), transposed ----
                nmx = softpool.tile([64, 1], f32, tag="nmx")
                nc.vector.tensor_reduce(
                    nmx[:], s2_sb[:], axis=mybir.AxisListType.X, op=Alu.max,
                    negate=True,
                )
                esum = softpool.tile([64, 1], f32, tag="esum")
                e_sb = softpool.tile([64, 64], f32, tag="E")
                nc.scalar.activation(
                    e_sb[:], s2_sb[:], Act.Exp, bias=nmx[:, 0:1], scale=1.0,
                    accum_out=esum[:],
                )
                rcp = softpool.tile([64, 1], f32, tag="rcp")
                nc.vector.reciprocal(rcp[:], esum[:])
                pi_sb = softpool.tile([64, 64], f32, tag="PI")
                # PI = (E * 1/sum) + I
                nc.vector.scalar_tensor_tensor(
                    pi_sb[:], e_sb[:], rcp[:, 0:1], ident[0:64, 0:64],
                    Alu.mult, Alu.add,
                )
                pit_ps = psP.tile([64, 64], f32, tag="PIT")
                nc.tensor.transpose(pit_ps[:], pi_sb[:], ident[0:64, 0:64])
                pit = softpool.tile([128, 64], f32, tag="PITb")
                nc.vector.tensor_copy(pit[0:64, :], pit_ps[:])
                nc.sync.dma_start(pit[64:128, :], pit[0:64, :])

                # ---- mm2: out = (P+I) @ X1, 128 windows of 512 cols ----
                # Output windows packed two-deep across PSUM/SBUF partition
                # halves (tile_position col groups) so stores run at full
                # 128-partition port width and mm2 matmuls pair up on the
                # two array column halves.
                for half in range(2):
                    lhs = pit[64 * half:64 * half + 64, :]
                    for g in range(8):            # groups of 8 windows (4096)
                        stg = stpool.tile([128, 4, 512], f32, tag="stage")
                        for hb in range(2):
                            for wi in range(4):
                                w = g * 8 + hb * 4 + wi
                                st = strips[w // 4]
                                rhs = st[64 * half:64 * half + 64,
                                         (w % 4) * 512:(w % 4) * 512 + 512]
                                o_ps = psO.tile([128, 512], f32, tag="O")
                                nc.tensor.matmul(
                                    o_ps[64 * hb:64 * hb + 64, :], lhs, rhs,
                                    start=True, stop=True,
                                )
                                if w % 2 == 0:
                                    nc.vector.tensor_copy(
                                        stg[64 * hb:64 * hb + 64, wi, :],
                                        o_ps[64 * hb:64 * hb + 64, :],
                                    )
                                else:
                                    nc.scalar.copy(
                                        stg[64 * hb:64 * hb + 64, wi, :],
                                        o_ps[64 * hb:64 * hb + 64, :],
                                    )
                        off = 32768 * half + g * 4096
                        nc.scalar.dma_start(
                            o1[:, off:off + 2048],
                            stg[0:64].rearrange("p a b -> p (a b)"),
                        )
                        nc.sync.dma_start(
                            o1[:, off + 2048:off + 4096],
                            stg[64:128].rearrange("p a b -> p (a b)"),
                        )

    nc.compile()
    return nc


def kernel(x: np.ndarray) -> np.ndarray:
    from concourse.bass_utils import run_bass_kernel_spmd

    if "nc" not in _CACHE:
        _CACHE["nc"] = _build()
    nc = _CACHE["nc"]

    x = np.ascontiguousarray(x, dtype=np.float32)
    B, Cc, H, W = x.shape
    xflat = x.reshape(B, Cc * H * W)
    in_maps = [
        {"xb": xflat[NB * i:NB * (i + 1)]} for i in range(NCORES)
    ]
    res = run_bass_kernel_spmd(nc, in_maps, core_ids=list(range(NCORES)))
    out = np.empty_like(xflat)
    for i in range(NCORES):
        out[NB * i:NB * (i + 1)] = res.results[i]["ob"]
    return out.reshape(B, Cc, H, W)



# revision 8
# speedup vs baseline: 1.8331x; 1.8331x over previous
"""ChannelAttention kernel for Trainium2 (8 NeuronCores, batch-parallel).

Reference computation per batch element b (C=64, N=H*W=65536):
    X1 = x[b] viewed [C, N]          (proj_query)
    X2 = x[b] viewed [N, C]          (proj_key -- a reshape, NOT a transpose)
    S  = X1 @ X2                     [C, C]
    P  = softmax(S, axis=-1)
    out[b] = (P @ X1) + X1  =  (P + I) @ X1

Sharding: data-parallel over batch. B=16 -> 2 batches per core on 8 cores.

Per-core dataflow (per batch):
  - x[b] resident in SBUF as 16 column-strips [128, 2048] f32: partition c
    holds X1[c, colhalf0-window], partition 64+c holds X1[c, colhalf1-window].
  - mm1 lhsT tiles: PE-transpose of strip slices [128,128] -> X1^T tiles for
    two n-windows at once (cols 0:64 = window u, cols 64:128 = window u+256).
  - mm1 rhs tiles: X2 contraction-major tiles streamed from HBM with a
    strided access pattern ([128, 32, 64] chunks per half), halves split
    across the two HWDGE queues. mm1 runs in fp32 (softmax is exponentially
    sensitive to absolute error in S; bf16 here fails the 2e-2 gate).
  - S accumulates over 512 fp32 matmuls into ONE [64, 64] PSUM tile:
    window u uses lhsT cols 0:64 vs V half 0, window u+256 uses cols 64:128
    vs V half 1 -- no diagonal-block fixup needed.
  - softmax: DVE row-max (negated) -> ACT exp with fused row-sum ->
    DVE reciprocal -> fused (E * 1/sum) + I; (P+I)^T via PE transpose,
    replicated to partitions 64:128 by a tiny DMA that overlaps mm2's
    first groups (which only read partitions 0:64 of pit... both halves
    actually -- the DMA is early and tiny either way).
  - mm2 in bf16 (1 cyc/row): strips are converted fp32->bf16 just-in-time
    on ACT (each bf16 strip lives ~1 group; 6-buf pool), matmuls
    [64p x 64] @ [64p x 512] -> PSUM fp32 -> DVE copy into [64, 2048] bf16
    staging -> 256KB stores. Output tensor is bf16; host converts back to
    f32 (bf16 rounding ~2e-3 rel err, tolerance is 2e-2). mm2 loops g
    outer / column-half inner so each bf16 strip is consumed right after
    conversion, and fp32 strips free early for the next batch's loads.
"""

import numpy as np

_CACHE = {}

B_FULL = 16
C = 64
N = 65536          # H*W = 256*256
NB = 2             # batches per core
NCORES = 8
NWIN = 256         # 128-col windows per column-half (32768 / 128)
NSTRIP = 16        # strips per batch; strip = [128, 2048]
STRIPW = 2048
VCH = 32           # X2 tiles per V chunk (1 MB per chunk)
NCHUNK = 8         # V chunk pairs per batch (8 * 32 = 256 windows)


def _build(loop_reps=1):
    from contextlib import nullcontext

    import concourse.bacc as bacc
    import concourse.mybir as mybir
    import concourse.tile as tile
    from concourse.masks import make_identity

    f32 = mybir.dt.float32
    bf16 = mybir.dt.bfloat16
    Alu = mybir.AluOpType
    Act = mybir.ActivationFunctionType

    nc = bacc.Bacc("TRN2", debug=False)
    xb = nc.dram_tensor("xb", [NB, C * N], f32, kind="ExternalInput").ap()
    ob = nc.dram_tensor("ob", [NB, C * N], bf16, kind="ExternalOutput").ap()

    with tile.TileContext(nc) as tc:
        with (
            tc.tile_pool(name="consts", bufs=1) as consts,
            tc.tile_pool(name="H", bufs=NSTRIP) as hpool,
            tc.tile_pool(name="V", bufs=2) as vpool,
            tc.tile_pool(name="B16", bufs=6) as bpool,
            tc.tile_pool(name="TOs", bufs=6) as topool,
            tc.tile_pool(name="stage", bufs=2) as stpool,
            tc.tile_pool(name="soft", bufs=2) as softpool,
            tc.tile_pool(name="psT", bufs=3, space="PSUM") as psT,
            tc.tile_pool(name="psS", bufs=2, space="PSUM") as psS,
            tc.tile_pool(name="psO", bufs=2, space="PSUM") as psO,
            tc.tile_pool(name="psP", bufs=1, space="PSUM") as psP,
        ):
            ident = consts.tile([128, 128], f32)
            make_identity(nc, ident[:])

            loop_cm = (
                tc.For_i(0, loop_reps, 1) if loop_reps > 1 else nullcontext()
            )
            with loop_cm:
              for b in range(NB):
                x1 = xb[b].rearrange("(c n) -> c n", c=C)      # [64, 65536]
                o1 = ob[b].rearrange("(c n) -> c n", c=C)

                # ---- load phase: interleave H strips and V chunks ----
                # Each strip is loaded by two 64-partition DMAs placed on the
                # two HWDGE rings (sync/scalar): partitions 0-63 hit the even
                # SBUF AXI ports and 64-127 the odd ones, so the concurrent
                # pair covers all 16 ports. V halves are likewise split
                # across both rings to balance descriptor generation.
                strips = []
                vtiles = []
                for j in range(NCHUNK):
                    for k in (2 * j, 2 * j + 1):
                        st = hpool.tile([128, STRIPW], f32, tag="H")
                        nc.sync.dma_start(
                            st[0:64, :], x1[:, k * STRIPW:(k + 1) * STRIPW]
                        )
                        nc.scalar.dma_start(
                            st[64:128, :],
                            x1[:, 32768 + k * STRIPW: 32768 + (k + 1) * STRIPW],
                        )
                        strips.append(st)
                    # V chunk: 16 window-pairs per half, layout
                    # [128, tl, half, 128] where partition p holds a 512B
                    # contiguous HBM run (rows 2p%... of the pair block), so
                    # descriptors are 512B and the load runs at line rate.
                    vt = vpool.tile([128, 16, 2, 128], f32, tag="V")
                    for half, eng in ((0, nc.sync), (1, nc.scalar)):
                        base = 2097152 * half + 262144 * j
                        src = xb[b][base:base + 262144].rearrange(
                            "(t p f) -> p t f", t=16, p=128, f=128
                        )
                        eng.dma_start(vt[:, :, half, :], src)
                    vtiles.append(vt)

                # ---- mm1: S = X1 @ X2, accumulated over 512 matmuls ----
                # Window u (u < 256): lhsT = transpose cols 0:64, rhs = V
                # half 0. Window u+256: lhsT = cols 64:128, rhs = V half 1.
                # All 512 accumulate into one [64, 64] PSUM tile.
                # PE stream is software-pipelined: transposes for pair p+SKEW
                # are emitted before the matmuls of pair p, so the PE never
                # waits on the PSUM->SBUF copy chain.
                SKEW = 2
                s_ps = psS.tile([64, 64], f32, tag="S")
                npairs = NWIN // 2               # 128 window pairs
                to_sbs = {}

                def emit_transpose(tp):
                    # Transpose input is a stride-2 interleaved view of the
                    # strip: output rows come out as [pair-window 2tp rows
                    # i=2v+q | window 2tp+1 rows i=2v+q], matching the
                    # contiguous V-tile partition order.
                    st = strips[tp // 8]
                    stv = st.rearrange(
                        "c (r w v q) -> c r w v q", r=8, w=2, v=64, q=2
                    )
                    r = tp % 8
                    to_ps = psT.tile([128, 2, 128], f32, tag="TO")
                    for q in range(2):
                        ti = stv[:, r, :, :, q]
                        nc.tensor.transpose(to_ps[:, q, :], ti, ident[:])
                    to_sb = topool.tile([128, 2, 128], f32, tag="TOs")
                    if tp % 2 == 0:
                        nc.scalar.copy(to_sb[:], to_ps[:])
                    else:
                        nc.vector.tensor_copy(to_sb[:], to_ps[:])
                    to_sbs[tp] = to_sb

                for tp in range(SKEW):
                    emit_transpose(tp)
                nmm = 0
                for tp in range(npairs):
                    if tp + SKEW < npairs:
                        emit_transpose(tp + SKEW)
                    to_sb = to_sbs.pop(tp)
                    j, tl = tp // 16, tp % 16
                    for q in range(2):
                        for half in range(2):
                            nc.tensor.matmul(
                                s_ps[:],
                                to_sb[:, q, 64 * half:64 * half + 64],
                                vtiles[j][:, tl, half,
                                          64 * q:64 * q + 64],
                                start=(nmm == 0), stop=(nmm == 2 * NWIN - 1),
                            )
                            nmm += 1

                # ---- softmax + (P + I), transposed ----
                nmx = softpool.tile([64, 1], f32, tag="nmx")
                nc.vector.tensor_reduce(
                    nmx[:], s_ps[:], axis=mybir.AxisListType.X, op=Alu.max,
                    negate=True,
                )
                esum = softpool.tile([64, 1], f32, tag="esum")
                e_sb = softpool.tile([64, 64], f32, tag="E")
                nc.scalar.activation(
                    e_sb[:], s_ps[:], Act.Exp, bias=nmx[:, 0:1], scale=1.0,
                    accum_out=esum[:],
                )
                rcp = softpool.tile([64, 1], f32, tag="rcp")
                nc.vector.reciprocal(rcp[:], esum[:])
                pi_sb = softpool.tile([64, 64], f32, tag="PI")
                # PI = (E * 1/sum) + I
                nc.vector.scalar_tensor_tensor(
                    pi_sb[:], e_sb[:], rcp[:, 0:1], ident[0:64, 0:64],
                    Alu.mult, Alu.add,
                )
                # (P+I)^T via PE transpose -> SBUF in bf16; replicate to
                # partitions 64:128 via tiny DMA.
                pit_ps = psP.tile([64, 64], f32, tag="PIT")
                nc.tensor.transpose(pit_ps[:], pi_sb[:], ident[0:64, 0:64])
                pit = softpool.tile([128, 64], bf16, tag="PITb")
                nc.vector.tensor_copy(pit[0:64, :], pit_ps[:])
                nc.sync.dma_start(pit[64:128, :], pit[0:64, :])

                # ---- mm2: out = (P+I) @ X1 in bf16 ----
                # Strips are converted to bf16 just-in-time on ACT; group g
                # uses bf16 strips 2g, 2g+1 for both column halves, then they
                # die. Stage copies all run on DVE so the ACT conversion
                # stream can never block work another group needs first.
                CONV_LA = 2          # groups of conversion lookahead

                def emit_conv(s):
                    sb = bpool.tile([128, STRIPW], bf16, tag="B16")
                    nc.scalar.copy(sb[:], strips[s][:])
                    return sb

                sbf = {}
                for s in range(2 * CONV_LA):
                    sbf[s] = emit_conv(s)
                for g in range(8):
                    for s in (2 * (g + CONV_LA), 2 * (g + CONV_LA) + 1):
                        if s < NSTRIP:
                            sbf[s] = emit_conv(s)
                    for half in range(2):
                        lhs = pit[64 * half:64 * half + 64, :]
                        stg = stpool.tile([128, 4, 512], bf16, tag="stage")
                        for wi in range(4):
                            o_ps = psO.tile([128, 512], f32, tag="O")
                            for hb in range(2):
                                w = g * 8 + hb * 4 + wi
                                sb = sbf[w // 4]
                                rhs = sb[64 * half:64 * half + 64,
                                         (w % 4) * 512:(w % 4) * 512 + 512]
                                nc.tensor.matmul(
                                    o_ps[64 * hb:64 * hb + 64, :], lhs, rhs,
                                    start=True, stop=True,
                                )
                            # full-width copy of both window halves at once
                            if wi % 2 == 0:
                                nc.vector.tensor_copy(stg[:, wi, :], o_ps[:])
                            else:
                                nc.scalar.copy(stg[:, wi, :], o_ps[:])
                        off = 32768 * half + g * 4096
                        nc.scalar.dma_start(
                            o1[:, off:off + 2048],
                            stg[0:64].rearrange("p a b -> p (a b)"),
                        )
                        nc.sync.dma_start(
                            o1[:, off + 2048:off + 4096],
                            stg[64:128].rearrange("p a b -> p (a b)"),
                        )

    nc.compile()
    return nc


def kernel(x: np.ndarray) -> np.ndarray:
    from concourse.bass_utils import run_bass_kernel_spmd

    if "nc" not in _CACHE:
        _CACHE["nc"] = _build()
    nc = _CACHE["nc"]

    x = np.ascontiguousarray(x, dtype=np.float32)
    B, Cc, H, W = x.shape
    xflat = x.reshape(B, Cc * H * W)
    in_maps = [
        {"xb": xflat[NB * i:NB * (i + 1)]} for i in range(NCORES)
    ]
    res = run_bass_kernel_spmd(nc, in_maps, core_ids=list(range(NCORES)))
    out = np.empty_like(xflat)
    for i in range(NCORES):
        out[NB * i:NB * (i + 1)] = res.results[i]["ob"].astype(np.float32)
    return out.reshape(B, Cc, H, W)
